# revision 1
# baseline (speedup 1.0000x reference)
"""Causal single-head attention (B=4, T=4096, C=1024, H=64) on 8 TRN2 NeuronCores.

Sharding: core = 2*b + p handles batch b and the 16 query/key row-blocks
(128 rows each) of parity p (block-cyclic over T for causal load balance).
Each core projects q/k/v for its own rows from a host-pretransposed x^T
slice, the core pair exchanges k^T/v^T per 512-column tile via AllGather,
and attention runs in the transposed orientation:
    S^T[s,t] = k^T.T @ q^T   (f32r matmuls, N=512 full rate)
    P^T = exp(S^T / 8)       (ACT, causality via 0/1 mask data per parity)
    out^T[h,t] = [v|1].T @ P^T  (row 64 accumulates softmax denominators)
then PE-transpose + normalize + DMA out.
"""
import numpy as np

import concourse.bacc as bacc
import concourse.bass as bass
import concourse.mybir as mybir
import concourse.tile as tile

dt = mybir.dt
F32R = dt.float32r
F32 = dt.float32

B, T, C, H = 4, 4096, 1024, 64
NBLK = T // 128            # 32 global blocks per batch
NLOC = NBLK // 2           # 16 blocks per core
NT = NLOC * 128            # 2048 query rows per core
NTT = NT // 512            # 4 t-tiles per core
N_CORES = 8
GROUPS = [[0, 1], [2, 3], [4, 5], [6, 7]]
SCALE = 1.0 / np.sqrt(H)

EXP = mybir.ActivationFunctionType.Exp


def _emit_body(nc, tc, aps, pools, rep):
    (xT_ap, wqk_ap, wv_ap, masks_ap, ident_ap, ones_ap, out_ap) = aps
    sb, ps, dr = pools

    # --- constants in SBUF ---
    wqk = sb.tile([128, 8 * 128], F32R, tag="wqk", name=f"wqk{rep}")
    wv = sb.tile([128, 8 * 64], F32R, tag="wv", name=f"wv{rep}")
    masks = sb.tile([128, 8 * 128], F32R, tag="masks", name=f"masks{rep}")
    identr = sb.tile([128, 128], F32R, tag="identr", name=f"identr{rep}")
    ident32 = sb.tile([128, 128], F32, tag="ident32", name=f"ident32{rep}")
    nc.sync.dma_start(wqk[:].rearrange("p (g h) -> p g h", g=8),
                      wqk_ap[:].rearrange("(g p) h -> p g h", p=128).bitcast(F32R))
    nc.sync.dma_start(wv[:].rearrange("p (g h) -> p g h", g=8),
                      wv_ap[:].rearrange("(g p) h -> p g h", p=128).bitcast(F32R))
    nc.sync.dma_start(masks[:], masks_ap[:].bitcast(F32R))
    nc.sync.dma_start(identr[:], ident_ap[:].bitcast(F32R))
    nc.sync.dma_start(ident32[:], ident_ap[:])

    # --- persistent activations ---
    qT_sb = sb.tile([64, NT], F32R, tag="qT", name=f"qT{rep}")
    kT_par = [sb.tile([64, NT], F32R, tag=f"kTp{j}", name=f"kTp{j}_{rep}") for j in (0, 1)]
    vT_par = [sb.tile([64, NT], F32R, tag=f"vTp{j}", name=f"vTp{j}_{rep}") for j in (0, 1)]
    vaug = [sb.tile([128, 65], F32R, tag=f"vaug{s}", name=f"vaug{s}_{rep}")
            for s in range(NBLK)]
    ones_sb = sb.tile([128, 1], F32R, tag="ones", name=f"ones{rep}")
    nc.sync.dma_start(ones_sb[:], ones_ap[:].bitcast(F32R))

    xT_3d = xT_ap[:].rearrange("(g p) n -> p g n", p=128)          # [128, 8, NT]

    # ---- stage A: projections + pair exchange + v_aug for one t-tile ----
    def emit_proj(tau):
        t0 = 512 * tau
        xt = sb.tile([128, 8 * 512], F32R, tag="xt", bufs=2, name=f"xt{rep}_{tau}")
        for c in range(8):  # chunked: matmuls start after the first 256KB
            nc.sync.dma_start(xt[:, 512*c:512*(c+1)],
                              xT_3d[:, c, t0:t0+512].bitcast(F32R))
        qkp = ps.tile([128, 512], F32, tag="qkp", name=f"qkp{rep}_{tau}")
        vp = ps.tile([64, 512], F32, tag="vp", name=f"vp{rep}_{tau}")
        for c in range(8):
            nc.tensor.matmul(qkp[:], wqk[:, 128*c:128*(c+1)], xt[:, 512*c:512*(c+1)],
                             start=(c == 0), stop=(c == 7))
        for c in range(8):
            nc.tensor.matmul(vp[:], wv[:, 64*c:64*(c+1)], xt[:, 512*c:512*(c+1)],
                             start=(c == 0), stop=(c == 7))
        kv = sb.tile([128, 512], F32R, tag="kv", bufs=2, name=f"kv{rep}_{tau}")
        nc.vector.tensor_copy(qT_sb[:, t0:t0+512], qkp[0:64, :])
        nc.vector.tensor_copy(kv[0:64, :], qkp[64:128, :])
        nc.vector.tensor_copy(kv[64:128, :], vp[:])

        ccin = dr.tile([128, 512], F32, tag="ccin", bufs=2, name=f"ccin{rep}_{tau}")
        ccout = dr.tile([2, 128, 512], F32, tag="ccout", bufs=2, name=f"ccout{rep}_{tau}")
        with tc.high_priority():
            nc.sync.dma_start(ccin[:], kv[:].bitcast(F32))
            if nc.num_devices > 1:
                nc.gpsimd.collective_compute(
                    "AllGather", mybir.AluOpType.bypass, replica_groups=GROUPS,
                    ins=[ccin[:]], outs=[ccout[:]],
                )
            else:  # single-core timing sim: stand-in DMAs with the same traffic
                nc.sync.dma_start(ccout[0], ccin[:])
                nc.sync.dma_start(ccout[1], ccin[:])
            for j in (0, 1):
                nc.sync.dma_start(kT_par[j][:, t0:t0+512], ccout[j, 0:64, :].bitcast(F32R))
                nc.sync.dma_start(vT_par[j][:, t0:t0+512], ccout[j, 64:128, :].bitcast(F32R))

        for s in range(8 * tau, 8 * tau + 8):
            tp = ps.tile([128, 64], F32R, tag="tr", name=f"trv{rep}_{s}")
            nc.tensor.transpose(tp[:], vT_par[s % 2][:, (s // 2)*128:(s // 2 + 1)*128],
                                identr[0:64, 0:64])
            nc.vector.tensor_copy(vaug[s][:, 0:64], tp[:])
            nc.vector.tensor_copy(vaug[s][:, 64:65], ones_sb[:])

    # ---- stage B: attention for one t-tile ----
    def emit_attn(tau):
        t0 = 512 * tau
        op = ps.tile([65, 512], F32, tag="outp", name=f"outp{rep}_{tau}")
        n_sig = 8 * tau + 8
        for m in range(n_sig // 2):
            s0, s1 = 2 * m, 2 * m + 1
            d0 = s0 - 8 * tau
            k = d0 // 2 if d0 >= 0 else 0
            off = 128 * k
            sp = ps.tile([128, 1024], F32, tag="sc", bufs=2, name=f"sc{rep}_{tau}_{m}")
            for idx, s in ((0, s0), (1, s1)):
                nc.tensor.matmul(
                    sp[:, 512*idx+off:512*(idx+1)],
                    kT_par[s % 2][:, (s // 2)*128:(s // 2 + 1)*128],
                    qT_sb[:, t0+off:t0+512],
                    start=True, stop=True)
            pt = sb.tile([128, 1024], F32R, tag="pt", bufs=3, name=f"pt{rep}_{tau}_{m}")
            sp_seg = sp[:].rearrange("p (s n) -> p s n", s=2)[:, :, off:512]
            pt_seg = pt[:].rearrange("p (s n) -> p s n", s=2)[:, :, off:512]
            nc.scalar.activation(pt_seg, sp_seg, EXP, scale=float(SCALE))
            if d0 >= 0:
                for idx, d in ((0, d0), (1, d0 + 1)):
                    seg = pt[:, 512*idx+off:512*idx+off+128]
                    nc.vector.tensor_mul(seg, seg, masks[:, 128*d:128*(d+1)])
            for idx, s in ((0, s0), (1, s1)):
                nc.tensor.matmul(
                    op[0:65, off:512], vaug[s][:], pt[:, 512*idx+off:512*(idx+1)],
                    start=(s == 0), stop=(s == n_sig - 1))

        # finalize: transpose back, normalize, one store per t-tile
        ob = sb.tile([65, 512], F32, tag="ob", bufs=2, name=f"ob{rep}_{tau}")
        nc.vector.tensor_copy(ob[:], op[:])
        ofin = sb.tile([128, 4 * 64], F32, tag="ofin", bufs=2, name=f"of{rep}_{tau}")
        for sub in range(4):
            tp2 = ps.tile([128, 65], F32, tag="tr", name=f"tr2{rep}_{tau}_{sub}")
            nc.tensor.transpose(tp2[:], ob[:, 128*sub:128*(sub+1)], ident32[0:65, 0:65])
            rc = sb.tile([128, 1], F32, tag="rc", bufs=2, name=f"rc{rep}_{tau}_{sub}")
            nc.vector.reciprocal(rc[:], tp2[:, 64:65])
            nc.vector.tensor_scalar_mul(ofin[:, 64*sub:64*(sub+1)], tp2[:, 0:64], rc[:])
        out_3d = out_ap[512*tau:512*(tau+1), :].rearrange("(s p) h -> p s h", p=128)
        nc.sync.dma_start(out_3d, ofin[:].rearrange("p (s h) -> p s h", s=4))

    # software pipeline: keep proj/exchange one t-tile ahead of attention
    for tau in range(NTT):
        emit_proj(tau)
        if tau >= 1:
            emit_attn(tau - 1)
    emit_attn(NTT - 1)


def build(reps=1, n_devices=N_CORES):
    nc = bacc.Bacc("TRN2", target_bir_lowering=False, debug=False,
                   num_devices=n_devices)
    xT_ap = nc.dram_tensor("xT", [C, NT], F32, kind="ExternalInput").ap()
    wqk_ap = nc.dram_tensor("wqk", [C, 128], F32, kind="ExternalInput").ap()
    wv_ap = nc.dram_tensor("wv", [C, 64], F32, kind="ExternalInput").ap()
    masks_ap = nc.dram_tensor("masks", [128, 8 * 128], F32, kind="ExternalInput").ap()
    ident_ap = nc.dram_tensor("ident", [128, 128], F32, kind="ExternalInput").ap()
    ones_ap = nc.dram_tensor("ones", [128, 1], F32, kind="ExternalInput").ap()
    out_ap = nc.dram_tensor("out", [NT, H], F32, kind="ExternalOutput").ap()
    aps = (xT_ap, wqk_ap, wv_ap, masks_ap, ident_ap, ones_ap, out_ap)

    with tile.TileContext(nc) as tc:
        with tc.tile_pool(name="sb", bufs=1) as sb, \
             tc.tile_pool(name="ps", bufs=1, space="PSUM") as ps, \
             tc.tile_pool(name="dr", bufs=1, space="DRAM") as dr:
            for rep in range(reps):
                _emit_body(nc, tc, aps, (sb, ps, dr), rep)
    nc.compile()
    return nc


def make_inputs(x, Wq, Wk, Wv):
    """Per-core input maps from full inputs."""
    x = np.asarray(x, dtype=np.float32)
    Wq, Wk, Wv = (np.asarray(w, dtype=np.float32) for w in (Wq, Wk, Wv))
    wqk = np.concatenate([Wq, Wk], axis=1)                      # [C, 128]
    tri = (np.arange(128)[:, None] <= np.arange(128)[None, :]).astype(np.float32)
    zeros = np.zeros((128, 128), np.float32)
    ones = np.ones((128, 128), np.float32)
    masks_even = np.concatenate([tri if d % 2 == 0 else zeros for d in range(8)], axis=1)
    masks_odd = np.concatenate([tri if d % 2 == 1 else ones for d in range(8)], axis=1)
    ident = np.eye(128, dtype=np.float32)
    ones_row = np.ones((128, 1), np.float32)

    in_maps = []
    for core in range(N_CORES):
        b, p = core // 2, core % 2
        xT = np.ascontiguousarray(
            x[b].T.reshape(C, NBLK, 128)[:, p::2, :].reshape(C, NT))
        in_maps.append({
            "xT": xT, "wqk": wqk, "wv": np.ascontiguousarray(Wv),
            "masks": masks_even if p == 0 else masks_odd,
            "ident": ident, "ones": ones_row,
        })
    return in_maps


def gather_output(results):
    """results: list per core of {"out": [NT, H]} → [B, T, H]."""
    out = np.empty((B, T, H), dtype=np.float32)
    for core in range(N_CORES):
        b, p = core // 2, core % 2
        o = results[core]["out"].reshape(NLOC, 128, H)
        out[b].reshape(NBLK, 128, H)[p::2] = o
    return out


# ---------------------------------------------------------------------------
# held PJRT runner (axon path) — inlined so kernel.py is self-contained
# ---------------------------------------------------------------------------

def make_runner(nc, n_cores):
    import jax
    from jax.sharding import Mesh, PartitionSpec
    from jax.experimental.shard_map import shard_map
    from concourse import bass2jax
    from concourse.bass2jax import _bass_exec_p, install_neuronx_cc_hook

    install_neuronx_cc_hook()
    partition_name = nc.partition_id_tensor.name if nc.partition_id_tensor else None

    in_names, out_names, out_avals, zero_shapes = [], [], [], []
    for alloc in nc.m.functions[0].allocations:
        if not isinstance(alloc, mybir.MemoryLocationSet):
            continue
        name = alloc.memorylocations[0].name
        if alloc.kind == "ExternalInput":
            if name != partition_name:
                in_names.append(name)
        elif alloc.kind == "ExternalOutput":
            out_names.append(name)
            shape = tuple(alloc.tensor_shape)
            dtype = mybir.dt.np(alloc.dtype)
            out_avals.append(jax.core.ShapedArray(shape, dtype))
            zero_shapes.append((shape, dtype))
    n_params, n_outs = len(in_names), len(out_avals)
    all_in_names = list(in_names) + list(out_names)
    if partition_name is not None:
        all_in_names.append(partition_name)
    donate = tuple(range(n_params, n_params + n_outs))

    def _body(*args):
        operands = list(args)
        if partition_name is not None:
            operands.append(bass2jax.partition_id_tensor())
        outs = _bass_exec_p.bind(
            *operands, out_avals=tuple(out_avals), in_names=tuple(all_in_names),
            out_names=tuple(out_names), lowering_input_output_aliases=(),
            sim_require_finite=True, sim_require_nnan=True, nc=nc)
        return tuple(outs)

    devices = jax.devices()[:n_cores]
    mesh = Mesh(np.asarray(devices), ("core",))
    sharded = jax.jit(
        shard_map(_body, mesh=mesh,
                  in_specs=(PartitionSpec("core"),) * (n_params + n_outs),
                  out_specs=(PartitionSpec("core"),) * n_outs, check_rep=False),
        donate_argnums=donate, keep_unused=True)
    make_zeros = jax.jit(lambda: tuple(
        jax.numpy.zeros((n_cores * s[0], *s[1:]), d) for (s, d) in zero_shapes))

    class Runner:
        def commit_inputs(self, in_maps):
            per_core = [[np.asarray(m[name]) for name in in_names] for m in in_maps]
            concat = [np.concatenate([per_core[c][i] for c in range(n_cores)], axis=0)
                      for i in range(n_params)]
            self._committed = [jax.device_put(a) for a in concat]
            jax.block_until_ready(self._committed)

        def run(self):
            outs = sharded(*self._committed, *make_zeros())
            jax.block_until_ready(outs)
            return outs

        def results(self, outs):
            res = [dict() for _ in range(n_cores)]
            for i, name in enumerate(out_names):
                per = np.split(np.asarray(outs[i]), n_cores, axis=0)
                for c in range(n_cores):
                    res[c][name] = per[c]
            return res

    return Runner()


_cache = {}


def get_runner(reps=1):
    if reps not in _cache:
        nc = build(reps)
        _cache[reps] = make_runner(nc, N_CORES)
    return _cache[reps]


def kernel(x, Wq, Wk, Wv):
    r = get_runner(1)
    r.commit_inputs(make_inputs(x, Wq, Wk, Wv))
    return gather_output(r.results(r.run()))

